# revision 19
# baseline (speedup 1.0000x reference)
"""Causal attention with relative keys (w = q@(k+k_r)^T/8, causal softmax,
returns (a, att)) on 8 Trainium2 NeuronCores.

Sharding: 2 batches x 16 heads = 32 (b,h) pairs -> 4 heads per core
(core c: b = c//4, heads 4*(c%4) .. 4*(c%4)+3). No cross-core comms.

Device computes, per head, the transposed score strips
    sT[k, q] = sum_d kT[d,k] * qT[d,q]        (q pre-scaled by 1/8 on host)
kj-strip-major (k-block index kj, q in [kj*128, 2048)), masks the diagonal
block, applies exp once (ACT) producing unnormalized probabilities P^T in
bf16, DMAs them straight out, and multiplies P^T with v (extended with a
ones column so row 64 of the result accumulates the softmax denominators).
Upper-triangle regions are never written: PJRT output buffers are donated
zero-filled, giving exact 0.0 there (matches exp(-1e10) underflow in the
reference).  The host divides by the denominators and rearranges layouts.
"""

import sys

import numpy as np

for _p in (
    "/root/.axon_site",
    "/root/.axon_site/_ro/trn_rl_repo",
    "/root/.axon_site/_ro/pypackages",
    "/opt/trn_rl_repo",
):
    if _p not in sys.path:
        sys.path.append(_p)

import ml_dtypes

BF16 = ml_dtypes.bfloat16

B, S, NX = 2, 2048, 1024
H, D = 16, 64
N_CORES = 8
HPC = H * B // N_CORES  # heads per core = 4
P = 128  # partition dim / k-block size
NKJ = S // P  # 16 k-blocks
QG = 512  # q-group width for the av matmuls
NQG = S // QG  # 4 q-groups
NEG_BIG = -1e10

_nc_cache = None


def _build_nc(S=S):
    import concourse.bacc as bacc
    import concourse.mybir as mybir
    import concourse.tile as tile

    F32 = mybir.dt.float32
    BF = mybir.dt.bfloat16
    EXP = mybir.ActivationFunctionType.Exp

    NKJ = S // P
    NQG = S // QG

    nc = bacc.Bacc(None, target_bir_lowering=False)

    # qT/kT ship as head-PAIR stacked tiles: [HPC//2, 2*D, S] with heads
    # 2i / 2i+1 on partitions 0-63 / 64-127 — enables PE row-tiling
    # (tile_position) so both heads' score matmuls run concurrently in
    # disjoint halves of the systolic array.
    qT_d = nc.dram_tensor("qT", [HPC // 2, 2 * D, S], BF, kind="ExternalInput")
    kT_d = nc.dram_tensor("kT", [HPC // 2, 2 * D, S], BF, kind="ExternalInput")
    v_d = nc.dram_tensor("vext", [P, NKJ, HPC, D + 1], BF, kind="ExternalInput")
    m_d = nc.dram_tensor("mask01", [P, P], BF, kind="ExternalInput")
    att_d = nc.dram_tensor("attT", [HPC, NKJ, P, S], BF, kind="ExternalOutput")
    a_d = nc.dram_tensor("avt", [HPC, NQG, D + 1, QG], F32, kind="ExternalOutput")

    with tile.TileContext(nc) as tc:
        with (
            tc.tile_pool(name="io", bufs=1) as io,
            tc.tile_pool(name="pt", bufs=1) as ptp,
            tc.tile_pool(name="ps", bufs=3, space="PSUM") as psp,
            tc.tile_pool(name="avp", bufs=2, space="PSUM") as avp,
            tc.tile_pool(name="ao", bufs=2) as aop,
        ):
            mask_sb = io.tile([P, P], BF, tag="mask")
            nc.sync.dma_start(mask_sb[:], m_d[:])
            # split q/k loads into 512-col chunks so the first score
            # matmuls unblock ~1us in instead of waiting for 2MB
            qT_sb, kT_sb = [], []
            for hp in range(HPC // 2):
                qt = io.tile([2 * D, S], BF, tag=f"q{hp}", name=f"qt{hp}")
                kt = io.tile([2 * D, S], BF, tag=f"k{hp}", name=f"kt{hp}")
                qT_sb.append(qt)
                kT_sb.append(kt)
            for hp in range(HPC // 2):
                for c0 in range(0, S, 512):
                    nc.sync.dma_start(
                        kT_sb[hp][:, c0 : c0 + 512], kT_d[hp, :, c0 : c0 + 512]
                    )
                    nc.sync.dma_start(
                        qT_sb[hp][:, c0 : c0 + 512], qT_d[hp, :, c0 : c0 + 512]
                    )
                if hp == 0:
                    v_sb = io.tile([P, NKJ, HPC, D + 1], BF, tag="v")
                    nc.sync.dma_start(v_sb[:], v_d[:])

            for hp in range(HPC // 2):  # head pair: heads 2hp, 2hp+1
                pts = {}  # (hh, kj) -> tile   (hh = 0/1 within pair)

                def emit_strips(hp, qg, pts=None):
                    # ---- pass A: score strips kj = 4*qg .. 4*qg+3,
                    #      both heads of the pair concurrently via
                    #      PE row-tiling ----
                    for kj in range(4 * qg, 4 * qg + 4):
                        ptA = ptp.tile([P, S], BF, tag=f"ptA{kj}")
                        ptB = ptp.tile([P, S], BF, tag=f"ptB{kj}")
                        pts[(0, kj)] = ptA
                        pts[(1, kj)] = ptB
                        c0 = kj * P
                        while c0 < S:
                            cw = min(1024, S - c0)
                            spsA = psp.tile([P, 1024], F32, tag="sps")
                            spsB = psp.tile([P, 1024], F32, tag="sps")
                            for n0 in range(0, cw, 512):
                                nw = min(512, cw - n0)
                                for hh, sps in ((0, spsA), (1, spsB)):
                                    nc.tensor.matmul(
                                        sps[:, n0 : n0 + nw],
                                        kT_sb[hp][
                                            hh * D : (hh + 1) * D,
                                            kj * P : (kj + 1) * P,
                                        ],
                                        qT_sb[hp][
                                            hh * D : (hh + 1) * D,
                                            c0 + n0 : c0 + n0 + nw,
                                        ],
                                        start=True,
                                        stop=True,
                                        tile_position=(hh * D, 0),
                                    )
                            for hh, sps, pt in ((0, spsA, ptA), (1, spsB, ptB)):
                                nc.scalar.activation(
                                    pt[:, c0 : c0 + cw], sps[:, 0:cw], EXP
                                )
                                if c0 == kj * P:
                                    # zero the strictly-masked (k>q) part of
                                    # the diagonal block (mask01 is 0 there)
                                    nc.vector.tensor_mul(
                                        pt[:, c0 : c0 + P],
                                        pt[:, c0 : c0 + P],
                                        mask_sb[:],
                                    )
                                nc.sync.dma_start(
                                    att_d[2 * hp + hh, kj, :, c0 : c0 + cw],
                                    pt[:, c0 : c0 + cw],
                                )
                            c0 += cw

                def emit_b(hp, qg, pts=None):
                    # ---- pass B: a-tiles for q-group qg (both heads) ----
                    q0 = qg * QG
                    last = 4 * qg + 3
                    for hh in range(2):
                        av = avp.tile([D + 1, QG], F32, tag="av")
                        for kj in range(0, last + 1):
                            off = max(0, kj * P - q0)
                            nc.tensor.matmul(
                                av[:, off:QG],
                                v_sb[:, kj, 2 * hp + hh, :],
                                pts[(hh, kj)][:, q0 + off : q0 + QG],
                                start=(kj == 0),
                                stop=(kj == last),
                            )
                        a_sb = aop.tile([D + 1, QG], F32, tag="ao")
                        nc.vector.tensor_copy(a_sb[:], av[:])
                        nc.sync.dma_start(a_d[2 * hp + hh, qg], a_sb[:])

                # strips lead pass-B by one q-group so PE always has
                # ready matmul work while ACT drains the current strips
                emit_strips(hp, 0, pts=pts)
                for qg in range(1, NQG):
                    emit_strips(hp, qg, pts=pts)
                    emit_b(hp, qg - 1, pts=pts)
                emit_b(hp, NQG - 1, pts=pts)
    nc.compile()
    return nc


def _get_nc():
    global _nc_cache
    if _nc_cache is None:
        _nc_cache = _build_nc()
    return _nc_cache


def _make_in_maps(query, key, value, key_r):
    qs = (np.asarray(query, np.float32) * 0.125).astype(np.float32)
    ksum = np.asarray(key, np.float32) + np.asarray(key_r, np.float32)
    val = np.asarray(value, np.float32)

    # mask01[p, c] = 1 where k<=q within the diagonal block (k=kj*128+p,
    # q=kj*128+c), 0 in the strictly-masked upper part (p > c).
    mask01 = np.triu(np.ones((P, P), np.float32)).astype(BF16)

    in_maps = []
    for c in range(N_CORES):
        b, h0 = c // HPC, (c % HPC) * HPC
        ch0 = h0 * D
        # [S, 4*D] -> [2, 2*D, S]  (head pairs stacked on partitions)
        qT = np.ascontiguousarray(
            qs[b, :, ch0 : ch0 + HPC * D].reshape(S, HPC // 2, 2 * D).transpose(1, 2, 0)
        ).astype(BF16)
        kT = np.ascontiguousarray(
            ksum[b, :, ch0 : ch0 + HPC * D]
            .reshape(S, HPC // 2, 2 * D)
            .transpose(1, 2, 0)
        ).astype(BF16)
        # [S, 4*D] -> [P, NKJ, 4, D] then append ones column
        vv = val[b, :, ch0 : ch0 + HPC * D].reshape(NKJ, P, HPC, D).transpose(1, 0, 2, 3)
        vext = np.concatenate(
            [vv, np.ones((P, NKJ, HPC, 1), np.float32)], axis=3
        ).astype(BF16)
        in_maps.append(
            {
                "qT": qT,
                "kT": kT,
                "vext": np.ascontiguousarray(vext),
                "mask01": mask01,
            }
        )
    return in_maps


def _run_device(in_maps, trace=False, **kw):
    from concourse.bass_utils import run_bass_kernel_spmd

    nc = _get_nc()
    return run_bass_kernel_spmd(
        nc, in_maps, core_ids=list(range(N_CORES)), trace=trace, **kw
    )


def _run_via_pjrt_fast(in_maps):
    """Like bass2jax.run_bass_via_pjrt, but the donated zero output
    buffers are materialized on-device (jnp.zeros under an explicit
    sharding) instead of being shipped from the host — saves ~264MB of
    tunnel transfer per call."""
    import jax
    import jax.numpy as jnp
    from jax.sharding import Mesh, NamedSharding, PartitionSpec
    from jax.experimental.shard_map import shard_map

    import concourse.mybir as mybir
    from concourse import bass2jax

    nc = _get_nc()
    bass2jax.install_neuronx_cc_hook()

    partition_name = nc.partition_id_tensor.name if nc.partition_id_tensor else None
    in_names, out_names, out_avals = [], [], []
    for alloc in nc.m.functions[0].allocations:
        if not isinstance(alloc, mybir.MemoryLocationSet):
            continue
        name = alloc.memorylocations[0].name
        if alloc.kind == "ExternalInput":
            if name != partition_name:
                in_names.append(name)
        elif alloc.kind == "ExternalOutput":
            out_names.append(name)
            out_avals.append(
                jax.core.ShapedArray(tuple(alloc.tensor_shape), mybir.dt.np(alloc.dtype))
            )
    n_params = len(in_names)
    n_outs = len(out_avals)
    all_in_names = list(in_names) + list(out_names)
    if partition_name is not None:
        all_in_names.append(partition_name)

    donate = tuple(range(n_params, n_params + n_outs))

    def _body(*args):
        operands = list(args)
        if partition_name is not None:
            operands.append(bass2jax.partition_id_tensor())
        outs = bass2jax._bass_exec_p.bind(
            *operands,
            out_avals=tuple(out_avals),
            in_names=tuple(all_in_names),
            out_names=tuple(out_names),
            lowering_input_output_aliases=(),
            sim_require_finite=True,
            sim_require_nnan=True,
            nc=nc,
        )
        return tuple(outs)

    devices = jax.devices()[:N_CORES]
    mesh = Mesh(np.asarray(devices), ("core",))
    spec = PartitionSpec("core")
    in_specs = (spec,) * (n_params + n_outs)
    out_specs = (spec,) * n_outs
    sharded = jax.jit(
        shard_map(_body, mesh=mesh, in_specs=in_specs, out_specs=out_specs,
                  check_rep=False),
        donate_argnums=donate,
        keep_unused=True,
    )
    concat_in = [
        np.concatenate([np.asarray(in_maps[c][name]) for c in range(N_CORES)], axis=0)
        for name in in_names
    ]
    sh = NamedSharding(mesh, spec)
    concat_zeros = [
        jax.jit(
            lambda av=av: jnp.zeros((N_CORES * av.shape[0], *av.shape[1:]), av.dtype),
            out_shardings=sh,
        )()
        for av in out_avals
    ]
    out_arrs = sharded(*concat_in, *concat_zeros)
    return [
        {
            name: np.asarray(out_arrs[i]).reshape(N_CORES, *out_avals[i].shape)[c]
            for i, name in enumerate(out_names)
        }
        for c in range(N_CORES)
    ]


def _assemble(results):
    att = np.empty((B, H, S, S), np.float32)
    a = np.empty((B, S, NX), np.float32)
    for c in range(N_CORES):
        b, h0 = c // HPC, (c % HPC) * HPC
        r = results[c]
        # attT [4, NKJ, P, S] bf16 -> [4, S(q), S(k)] f32
        att_part = (
            r["attT"].astype(np.float32).transpose(0, 3, 1, 2).reshape(HPC, S, S)
        )
        avt = r["avt"]  # [4, NQG, D+1, QG] f32
        sums = avt[:, :, D, :].reshape(HPC, S)  # [4, S]
        att_part /= sums[:, :, None]
        att[b, h0 : h0 + HPC] = att_part
        # av [4, NQG, D, QG] -> [S, 4, D]
        av = avt[:, :, :D, :].transpose(1, 3, 0, 2).reshape(S, HPC, D)
        av /= sums.T[:, :, None]
        a[b, :, h0 * D : (h0 + HPC) * D] = av.reshape(S, HPC * D)
    return a, att


def kernel(query, key, value, key_r):
    in_maps = _make_in_maps(query, key, value, key_r)
    try:
        results = _run_via_pjrt_fast(in_maps)
    except Exception:
        results = _run_device(in_maps, trace=False).results
    return _assemble(results)


# revision 21
# speedup vs baseline: 1.0410x; 1.0410x over previous
"""Causal attention with relative keys (w = q@(k+k_r)^T/8, causal softmax,
returns (a, att)) on 8 Trainium2 NeuronCores.

Sharding: 2 batches x 16 heads = 32 (b,h) pairs -> 4 heads per core
(core c: b = c//4, heads 4*(c%4) .. 4*(c%4)+3). No cross-core comms.

Device computes, per head, the transposed score strips
    sT[k, q] = sum_d kT[d,k] * qT[d,q]        (q pre-scaled by 1/8 on host)
kj-strip-major (k-block index kj, q in [kj*128, 2048)), masks the diagonal
block, applies exp once (ACT) producing unnormalized probabilities P^T in
bf16, DMAs them straight out, and multiplies P^T with v (extended with a
ones column so row 64 of the result accumulates the softmax denominators).
Upper-triangle regions are never written: PJRT output buffers are donated
zero-filled, giving exact 0.0 there (matches exp(-1e10) underflow in the
reference).  The host divides by the denominators and rearranges layouts.
"""

import sys

import numpy as np

for _p in (
    "/root/.axon_site",
    "/root/.axon_site/_ro/trn_rl_repo",
    "/root/.axon_site/_ro/pypackages",
    "/opt/trn_rl_repo",
):
    if _p not in sys.path:
        sys.path.append(_p)

import ml_dtypes

BF16 = ml_dtypes.bfloat16

B, S, NX = 2, 2048, 1024
H, D = 16, 64
N_CORES = 8
HPC = H * B // N_CORES  # heads per core = 4
P = 128  # partition dim / k-block size
NKJ = S // P  # 16 k-blocks
QG = 512  # q-group width for the av matmuls
NQG = S // QG  # 4 q-groups
NEG_BIG = -1e10

_nc_cache = None


def _build_nc(S=S):
    import concourse.bacc as bacc
    import concourse.mybir as mybir
    import concourse.tile as tile

    F32 = mybir.dt.float32
    BF = mybir.dt.bfloat16
    EXP = mybir.ActivationFunctionType.Exp

    NKJ = S // P
    NQG = S // QG

    nc = bacc.Bacc(None, target_bir_lowering=False)

    # qT/kT ship as head-PAIR stacked tiles: [HPC//2, 2*D, S] with heads
    # 2i / 2i+1 on partitions 0-63 / 64-127 — enables PE row-tiling
    # (tile_position) so both heads' score matmuls run concurrently in
    # disjoint halves of the systolic array.
    qT_d = nc.dram_tensor("qT", [HPC // 2, 2 * D, S], BF, kind="ExternalInput")
    kT_d = nc.dram_tensor("kT", [HPC // 2, 2 * D, S], BF, kind="ExternalInput")
    v_d = nc.dram_tensor("vext", [P, NKJ, HPC, D + 1], BF, kind="ExternalInput")
    m_d = nc.dram_tensor("mask01", [P, P], BF, kind="ExternalInput")
    att_d = nc.dram_tensor("attT", [HPC, NKJ, P, S], BF, kind="ExternalOutput")
    a_d = nc.dram_tensor("avt", [HPC, NQG, D + 1, QG], F32, kind="ExternalOutput")

    with tile.TileContext(nc) as tc:
        with (
            tc.tile_pool(name="io", bufs=1) as io,
            tc.tile_pool(name="pt", bufs=1) as ptp,
            tc.tile_pool(name="ps", bufs=3, space="PSUM") as psp,
            tc.tile_pool(name="avp", bufs=2, space="PSUM") as avp,
            tc.tile_pool(name="ao", bufs=2) as aop,
        ):
            mask_sb = io.tile([P, P], BF, tag="mask")
            nc.sync.dma_start(mask_sb[:], m_d[:])
            qT_sb, kT_sb = [], []
            for hp in range(HPC // 2):
                kt = io.tile([2 * D, S], BF, tag=f"k{hp}", name=f"kt{hp}")
                nc.sync.dma_start(kt[:], kT_d[hp])
                qt = io.tile([2 * D, S], BF, tag=f"q{hp}", name=f"qt{hp}")
                nc.sync.dma_start(qt[:], qT_d[hp])
                qT_sb.append(qt)
                kT_sb.append(kt)
                if hp == 0:
                    v_sb = io.tile([P, NKJ, HPC, D + 1], BF, tag="v")
                    nc.sync.dma_start(v_sb[:], v_d[:])

            for hp in range(HPC // 2):  # head pair: heads 2hp, 2hp+1
                pts = {}  # (hh, kj) -> tile   (hh = 0/1 within pair)

                def emit_strip(hp, kj, pts):
                    # score strip kj (q in [kj*P, S)) for both heads of
                    # the pair concurrently via PE row-tiling
                    ptA = ptp.tile([P, S], BF, tag=f"ptA{kj}")
                    ptB = ptp.tile([P, S], BF, tag=f"ptB{kj}")
                    pts[(0, kj)] = ptA
                    pts[(1, kj)] = ptB
                    c0 = kj * P
                    while c0 < S:
                        cw = min(1024, S - c0)
                        spsA = psp.tile([P, 1024], F32, tag="sps")
                        spsB = psp.tile([P, 1024], F32, tag="sps")
                        for n0 in range(0, cw, 512):
                            nw = min(512, cw - n0)
                            for hh, sps in ((0, spsA), (1, spsB)):
                                nc.tensor.matmul(
                                    sps[:, n0 : n0 + nw],
                                    kT_sb[hp][
                                        hh * D : (hh + 1) * D, kj * P : (kj + 1) * P
                                    ],
                                    qT_sb[hp][
                                        hh * D : (hh + 1) * D, c0 + n0 : c0 + n0 + nw
                                    ],
                                    start=True,
                                    stop=True,
                                    tile_position=(hh * D, 0),
                                )
                        for hh, sps, pt in ((0, spsA, ptA), (1, spsB, ptB)):
                            nc.scalar.activation(pt[:, c0 : c0 + cw], sps[:, 0:cw], EXP)
                            if c0 == kj * P:
                                # zero the strictly-masked (k>q) part of
                                # the diagonal block (mask01 is 0 there)
                                nc.vector.tensor_mul(
                                    pt[:, c0 : c0 + P], pt[:, c0 : c0 + P], mask_sb[:]
                                )
                        c0 += cw
                    for hh, pt in ((0, ptA), (1, ptB)):
                        # one whole-strip DMA: contiguous rows, fewer
                        # descriptors than per-chunk stores
                        nc.sync.dma_start(
                            att_d[2 * hp + hh, kj, :, kj * P : S], pt[:, kj * P : S]
                        )

                for qg in range(NQG):
                    q0 = qg * QG
                    last = 4 * qg + 3
                    navm = last + 1  # av matmuls per head for this group
                    avs = {}
                    for hh in range(2):
                        avs[hh] = avp.tile([D + 1, QG], F32, tag="av", name=f"av{hh}")
                    for j in range(4):  # strips of this group
                        emit_strip(hp, 4 * qg + j, pts)
                        # interleave this group's av matmuls batch-wise:
                        # after strip j, emit kj batch [ceil(navm*j/4),
                        # ceil(navm*(j+1)/4)) — each batch only uses
                        # strips that are already computed (kj <= 4qg+j)
                        b0 = -(-navm * j // 4)
                        b1 = -(-navm * (j + 1) // 4)
                        for kj in range(b0, b1):
                            for hh in range(2):
                                off = max(0, kj * P - q0)
                                nc.tensor.matmul(
                                    avs[hh][:, off:QG],
                                    v_sb[:, kj, 2 * hp + hh, :],
                                    pts[(hh, kj)][:, q0 + off : q0 + QG],
                                    start=(kj == 0),
                                    stop=(kj == last),
                                )
                    for hh in range(2):
                        a_sb = aop.tile([D + 1, QG], F32, tag="ao")
                        nc.vector.tensor_copy(a_sb[:], avs[hh][:])
                        nc.sync.dma_start(a_d[2 * hp + hh, qg], a_sb[:])
    nc.compile()
    return nc


def _get_nc():
    global _nc_cache
    if _nc_cache is None:
        _nc_cache = _build_nc()
    return _nc_cache


def _make_in_maps(query, key, value, key_r):
    qs = (np.asarray(query, np.float32) * 0.125).astype(np.float32)
    ksum = np.asarray(key, np.float32) + np.asarray(key_r, np.float32)
    val = np.asarray(value, np.float32)

    # mask01[p, c] = 1 where k<=q within the diagonal block (k=kj*128+p,
    # q=kj*128+c), 0 in the strictly-masked upper part (p > c).
    mask01 = np.triu(np.ones((P, P), np.float32)).astype(BF16)

    in_maps = []
    for c in range(N_CORES):
        b, h0 = c // HPC, (c % HPC) * HPC
        ch0 = h0 * D
        # [S, 4*D] -> [2, 2*D, S]  (head pairs stacked on partitions)
        qT = np.ascontiguousarray(
            qs[b, :, ch0 : ch0 + HPC * D].reshape(S, HPC // 2, 2 * D).transpose(1, 2, 0)
        ).astype(BF16)
        kT = np.ascontiguousarray(
            ksum[b, :, ch0 : ch0 + HPC * D]
            .reshape(S, HPC // 2, 2 * D)
            .transpose(1, 2, 0)
        ).astype(BF16)
        # [S, 4*D] -> [P, NKJ, 4, D] then append ones column
        vv = val[b, :, ch0 : ch0 + HPC * D].reshape(NKJ, P, HPC, D).transpose(1, 0, 2, 3)
        vext = np.concatenate(
            [vv, np.ones((P, NKJ, HPC, 1), np.float32)], axis=3
        ).astype(BF16)
        in_maps.append(
            {
                "qT": qT,
                "kT": kT,
                "vext": np.ascontiguousarray(vext),
                "mask01": mask01,
            }
        )
    return in_maps


def _run_device(in_maps, trace=False, **kw):
    from concourse.bass_utils import run_bass_kernel_spmd

    nc = _get_nc()
    return run_bass_kernel_spmd(
        nc, in_maps, core_ids=list(range(N_CORES)), trace=trace, **kw
    )


def _run_via_pjrt_fast(in_maps):
    """Like bass2jax.run_bass_via_pjrt, but the donated zero output
    buffers are materialized on-device (jnp.zeros under an explicit
    sharding) instead of being shipped from the host — saves ~264MB of
    tunnel transfer per call."""
    import jax
    import jax.numpy as jnp
    from jax.sharding import Mesh, NamedSharding, PartitionSpec
    from jax.experimental.shard_map import shard_map

    import concourse.mybir as mybir
    from concourse import bass2jax

    nc = _get_nc()
    bass2jax.install_neuronx_cc_hook()

    partition_name = nc.partition_id_tensor.name if nc.partition_id_tensor else None
    in_names, out_names, out_avals = [], [], []
    for alloc in nc.m.functions[0].allocations:
        if not isinstance(alloc, mybir.MemoryLocationSet):
            continue
        name = alloc.memorylocations[0].name
        if alloc.kind == "ExternalInput":
            if name != partition_name:
                in_names.append(name)
        elif alloc.kind == "ExternalOutput":
            out_names.append(name)
            out_avals.append(
                jax.core.ShapedArray(tuple(alloc.tensor_shape), mybir.dt.np(alloc.dtype))
            )
    n_params = len(in_names)
    n_outs = len(out_avals)
    all_in_names = list(in_names) + list(out_names)
    if partition_name is not None:
        all_in_names.append(partition_name)

    donate = tuple(range(n_params, n_params + n_outs))

    def _body(*args):
        operands = list(args)
        if partition_name is not None:
            operands.append(bass2jax.partition_id_tensor())
        outs = bass2jax._bass_exec_p.bind(
            *operands,
            out_avals=tuple(out_avals),
            in_names=tuple(all_in_names),
            out_names=tuple(out_names),
            lowering_input_output_aliases=(),
            sim_require_finite=True,
            sim_require_nnan=True,
            nc=nc,
        )
        return tuple(outs)

    devices = jax.devices()[:N_CORES]
    mesh = Mesh(np.asarray(devices), ("core",))
    spec = PartitionSpec("core")
    in_specs = (spec,) * (n_params + n_outs)
    out_specs = (spec,) * n_outs
    sharded = jax.jit(
        shard_map(_body, mesh=mesh, in_specs=in_specs, out_specs=out_specs,
                  check_rep=False),
        donate_argnums=donate,
        keep_unused=True,
    )
    concat_in = [
        np.concatenate([np.asarray(in_maps[c][name]) for c in range(N_CORES)], axis=0)
        for name in in_names
    ]
    sh = NamedSharding(mesh, spec)
    concat_zeros = [
        jax.jit(
            lambda av=av: jnp.zeros((N_CORES * av.shape[0], *av.shape[1:]), av.dtype),
            out_shardings=sh,
        )()
        for av in out_avals
    ]
    out_arrs = sharded(*concat_in, *concat_zeros)
    return [
        {
            name: np.asarray(out_arrs[i]).reshape(N_CORES, *out_avals[i].shape)[c]
            for i, name in enumerate(out_names)
        }
        for c in range(N_CORES)
    ]


def _assemble(results):
    att = np.empty((B, H, S, S), np.float32)
    a = np.empty((B, S, NX), np.float32)
    for c in range(N_CORES):
        b, h0 = c // HPC, (c % HPC) * HPC
        r = results[c]
        # attT [4, NKJ, P, S] bf16 -> [4, S(q), S(k)] f32
        att_part = (
            r["attT"].astype(np.float32).transpose(0, 3, 1, 2).reshape(HPC, S, S)
        )
        avt = r["avt"]  # [4, NQG, D+1, QG] f32
        sums = avt[:, :, D, :].reshape(HPC, S)  # [4, S]
        att_part /= sums[:, :, None]
        att[b, h0 : h0 + HPC] = att_part
        # av [4, NQG, D, QG] -> [S, 4, D]
        av = avt[:, :, :D, :].transpose(1, 3, 0, 2).reshape(S, HPC, D)
        av /= sums.T[:, :, None]
        a[b, :, h0 * D : (h0 + HPC) * D] = av.reshape(S, HPC * D)
    return a, att


def kernel(query, key, value, key_r):
    in_maps = _make_in_maps(query, key, value, key_r)
    try:
        results = _run_via_pjrt_fast(in_maps)
    except Exception:
        results = _run_device(in_maps, trace=False).results
    return _assemble(results)
